# revision 18
# baseline (speedup 1.0000x reference)
"""Correlation kernel (max_disp=1, 9 offsets) for Trainium2, 8 NeuronCores.

Computation (per batch b):
    out[dx*3+dy, i, j] = mean_c( x1[c,i,j] * pad(x2)[c, i+dy, j+dx] )
with B=8, C=512, H=W=96, pad=1 on each spatial side.

Sharding: data-parallel over batch — core b handles batch b. No collectives.

Per-core strategy (v3):
  - C (512) on SBUF partitions, 4 chunks of 128; spatial (96x96=9216) on free dim.
  - Inputs DMA'd with fp32->bf16 cast (SWDGE, gpsimd triggers), split into
    row-halves so compute starts after ~half the first chunk's bytes land.
  - x2 lands in a zero-padded [128, 98, 98] tile; two flat-shifted copies
    (od0 rows 0..49, od1 rows 48..97; SBUF->SBUF DMA on the SP HWDGE ring)
    keep dx=1 views 4B-aligned so DVE tensor_mul runs in 2x mode.
  - Products are computed per row-half ([128, 48, 96] bf16, 72 of them),
    split between DVE (2x mode) and GpSimd/Pool (slow but otherwise idle).
    Pool gets dx!=1 offsets only (no od dependency); DVE does od-based
    offsets last.
  - TensorE reduces over partitions via matmuls with a 32-wide sliding
    one-hot-column stationary (LDWEIGHTS = 32 cols): offset k, global block
    bb (= 9*half + blk) -> row m = 18k + bb -> PSUM bank m//32, row m%32.
    Six [32, 512] PSUM tiles (one bank each); every MM writes the full tile
    so one start=True per bank is safe with bank-granular has_written.
  - Act engine scales PSUM banks (in completion order) by 1/512 into SBUF
    and issues the output DMAs.
"""

import os
import sys

for _p in ("/opt/trn_rl_repo",):
    if os.path.isdir(_p) and _p not in sys.path:
        sys.path.insert(0, _p)

from contextlib import ExitStack

import numpy as np

import concourse.bass as bass
import concourse.mybir as mybir
import concourse.tile as tile
from concourse import bacc
from concourse.bass_utils import run_bass_kernel_spmd

B, C, H, W = 8, 512, 96, 96
NCORES = 8
PW = W + 2          # padded spatial width
HH = H // 2         # 48 rows per half
NCHUNK = C // 128   # 4
NBLK = HH * W // 512  # 9 512-col blocks per half
F32 = mybir.dt.float32
BF16 = mybir.dt.bfloat16


# (ch, k) product tiles computed on GpSimd (Pool) instead of DVE. Only dx!=1
# offsets (k not in 3..5) so Pool never waits on the od copies.
def _pool_set():
    n = int(os.environ.get("CORR_POOL_MULS", "5"))
    cand = [(0, 0), (0, 8), (1, 0), (2, 0), (3, 0), (1, 8), (2, 8), (3, 8)]
    return set(cand[:n])


def _corr_body(ctx: ExitStack, tc: "tile.TileContext", out_t, x1_t, x2_t, nchunk=NCHUNK):
    nc = tc.nc
    pool_set = _pool_set()

    wpool = ctx.enter_context(tc.tile_pool(name="wm", bufs=1))
    x1pool = ctx.enter_context(tc.tile_pool(name="x1", bufs=2))
    evpool = ctx.enter_context(tc.tile_pool(name="ev", bufs=3))
    odpool = ctx.enter_context(tc.tile_pool(name="od", bufs=2))
    prpool = ctx.enter_context(tc.tile_pool(name="pr", bufs=int(os.environ.get("CORR_PROD_BUFS", "5"))))
    pppool = ctx.enter_context(tc.tile_pool(name="pp", bufs=int(os.environ.get("CORR_PPROD_BUFS", "2"))))
    pspool = ctx.enter_context(
        tc.tile_pool(name="ps", bufs=1, space=bass.MemorySpace.PSUM)
    )
    outpool = ctx.enter_context(tc.tile_pool(name="ot", bufs=1))

    # Sliding 32-wide one-hot stationaries: slice [:, s : s+32] has its all-ones
    # column at local position r when s = ones_col - r. Two masters (ones at
    # cols 30 and 31) keep s even for either parity of r, so every LDWEIGHTS
    # source is 4-byte aligned (bf16).
    wmE = wpool.tile([128, 64], BF16)
    nc.vector.memset(wmE[:, :], 0.0)
    nc.vector.memset(wmE[:, 30:31], 1.0)
    wmO = wpool.tile([128, 64], BF16)
    nc.vector.memset(wmO[:, :], 0.0)
    nc.vector.memset(wmO[:, 31:32], 1.0)

    def wslice(r: int):
        wm, col = (wmE, 30) if r % 2 == 0 else (wmO, 31)
        s = col - r
        return wm[:, s : s + 32]

    # 6 PSUM accumulators, one bank each, 32 rows used per bank (162 rows
    # total). Every MM writes the full [32, 512] tile (one-hot row gets the
    # sum, the rest accumulate zeros), so a single start=True per bank works
    # with the bank-granular has_written clear.
    ps = [pspool.tile([32, 512], F32, name=f"ps{t}") for t in range(6)]

    x1f = x1_t.ap()  # [512, 96, 96] f32 DRAM
    x2f = x2_t.ap()

    def ev_ks(ch):
        return [k for k in (0, 1, 2, 6, 7, 8) if (ch, k) not in pool_set]

    def pool_ks(ch):
        return [k for k in (0, 1, 2, 6, 7, 8) if (ch, k) in pool_set]

    OD_KS = [3, 4, 5]

    # Per-chunk phase sequence: ("dve", k, row0, nrows) products (+ their MMs)
    # and ("pool_mm", k, h) MM groups for Pool-computed half products. Chunk 0
    # runs its ev-based products in thirds (32 rows) so compute starts as soon
    # as the first third of the first chunk's bytes lands.
    def phases(ch):
        if ch == 0:
            return (
                [("dve", k, 0, 32) for k in ev_ks(ch)]
                + [("dve", k, 32, 32) for k in ev_ks(ch)]
                + [("dve", k, 0, 48) for k in OD_KS]
                + [("pool_mm", k, 0) for k in pool_ks(ch)]
                + [("dve", k, 64, 32) for k in ev_ks(ch)]
                + [("dve", k, 48, 48) for k in OD_KS]
                + [("pool_mm", k, 1) for k in pool_ks(ch)]
            )
        return (
            [("dve", k, 0, 48) for k in ev_ks(ch)]
            + [("dve", k, 0, 48) for k in OD_KS]
            + [("pool_mm", k, 0) for k in pool_ks(ch)]
            + [("dve", k, 48, 48) for k in ev_ks(ch)]
            + [("dve", k, 48, 48) for k in OD_KS]
            + [("pool_mm", k, 1) for k in pool_ks(ch)]
        )

    # Emission-order plan of all MMs: (ch, k, gb) with gb the global 512-col
    # block index (row0*3//16 + j); PSUM row m = 18k + gb, bank q = m//32.
    mm_plan = []
    for ch in range(nchunk):
        for item in phases(ch):
            if item[0] == "dve":
                _, k, row0, nrows = item
            else:
                _, k, h = item
                row0, nrows = 48 * h, 48
            for j in range(nrows * 3 // 16):
                mm_plan.append((ch, k, row0 * 3 // 16 + j))
    last_mm_for_bank = {}
    bank_completion = []
    for ch, k, gb in mm_plan:
        q = (18 * k + gb) // 32
        last_mm_for_bank[q] = (ch, k, gb)
        if q in bank_completion:
            bank_completion.remove(q)
        bank_completion.append(q)

    started = [False] * 6

    x1bf = [None] * nchunk
    ev = [None] * nchunk
    od0 = [None] * nchunk
    od1 = [None] * nchunk

    def emit_ev_tile(ch):
        t = evpool.tile([128, PW, PW], BF16, name="ev")
        ev[ch] = t
        # borders on the gpsimd stream (cheap; WAR-free with ev bufs=3)
        nc.gpsimd.memset(t[:, 0, :], 0.0)
        nc.gpsimd.memset(t[:, PW - 1, :], 0.0)
        nc.gpsimd.memset(t[:, 1 : PW - 1, 0], 0.0)
        nc.gpsimd.memset(t[:, 1 : PW - 1, PW - 1], 0.0)
        return t

    def emit_ev_dma(ch, r0, r1):
        p0 = ch * 128
        nc.gpsimd.dma_start(
            out=ev[ch][:, 1 + r0 : 1 + r1, 1 : PW - 1],
            in_=x2f[p0 : p0 + 128, r0:r1, :],
        )

    def emit_x1_tile(ch):
        x1bf[ch] = x1pool.tile([128, H, W], BF16, name="x1bf")

    def emit_x1_dma(ch, r0, r1):
        p0 = ch * 128
        nc.gpsimd.dma_start(
            out=x1bf[ch][:, r0:r1, :], in_=x1f[p0 : p0 + 128, r0:r1, :]
        )

    def emit_od_copies(ch):
        ev_flat = ev[ch][:, :, :].rearrange("p a b -> p (a b)")
        # odd copies: flat shift-by-one so dx=1 views are 4B-aligned for the
        # DVE 2x mode. Copied on the otherwise-idle Act engine (alignment-
        # agnostic, no DMA-queue or SBUF-fabric contention).
        # od0 covers padded rows 0..49, od1 rows 48..97 (2-row overlap).
        o0 = odpool.tile([128, 50, PW], BF16, name="od0")
        od0[ch] = o0
        o0_flat = o0[:, :, :].rearrange("p a b -> p (a b)")
        nc.scalar.copy(o0_flat[:, 0 : 50 * PW], ev_flat[:, 1 : 50 * PW + 1])
        o1 = odpool.tile([128, 50, PW], BF16, name="od1")
        od1[ch] = o1
        o1_flat = o1[:, :, :].rearrange("p a b -> p (a b)")
        nc.scalar.copy(
            o1_flat[:, 0 : 50 * PW - 1], ev_flat[:, 48 * PW + 1 : PW * PW]
        )

    def emit_loads_head(ch, cuts):
        # interleaved ev/x1 loads for the first two chunks (head latency)
        emit_ev_tile(ch)
        emit_x1_tile(ch)
        evcuts = [0] + [c + 2 for c in cuts[1:-1]] + [H]  # ev needs 2 extra rows
        for i in range(len(cuts) - 1):
            emit_ev_dma(ch, evcuts[i], evcuts[i + 1])
            emit_x1_dma(ch, cuts[i], cuts[i + 1])
        emit_od_copies(ch)

    def emit_ev_loads(ch):
        emit_ev_tile(ch)
        emit_ev_dma(ch, 0, 50)
        emit_ev_dma(ch, 50, H)
        emit_od_copies(ch)

    def emit_x1_loads(ch):
        emit_x1_tile(ch)
        emit_x1_dma(ch, 0, HH)
        emit_x1_dma(ch, HH, H)

    def view_for(ch, k, row0, nrows):
        dx, dy = k // 3, k % 3
        if dx == 1:
            assert (row0, nrows) in ((0, 48), (48, 48))
            src = od0[ch] if row0 == 0 else od1[ch]
            return src[:, dy : dy + 48, 0:W]
        return ev[ch][:, row0 + dy : row0 + dy + nrows, dx : dx + W]

    def emit_mms(ch, k, row0, nrows, prod):
        prod_flat = prod[:, :, :].rearrange("p a b -> p (a b)")
        gb0 = row0 * 3 // 16
        for j in range(nrows * 3 // 16):
            gb = gb0 + j
            m = 18 * k + gb
            q, r = m // 32, m % 32
            st = not started[q]
            started[q] = True
            last = last_mm_for_bank[q] == (ch, k, gb)
            nc.tensor.matmul(
                ps[q][:, :],
                wslice(r),
                prod_flat[:, j * 512 : (j + 1) * 512],
                start=st,
                stop=last,
            )

    emit_loads_head(0, [0, 32, 64, H])
    if nchunk > 1:
        emit_loads_head(1, [0, HH, H])

    for ch in range(nchunk):
        # ev loads for chunk ch+2: WAR-free with ev bufs=3, so the triggers
        # fire immediately and never block the Pool products queued behind.
        if ch + 2 < nchunk:
            emit_ev_loads(ch + 2)
        # Pool products for this chunk (long-running; start early).
        prods = {}
        for h in range(2):
            for k in pool_ks(ch):
                prod = pppool.tile([128, HH, W], BF16, name="pprod")
                prods[(k, h)] = prod
                nc.gpsimd.tensor_mul(
                    prod[:, :, :],
                    x1bf[ch][:, 48 * h : 48 * h + HH, :],
                    view_for(ch, k, 48 * h, 48),
                )
        # x1 loads for ch+2 after this chunk's Pool products: the WAR wait
        # (x1 bufs=2, readers = chunk ch's products) sits behind them on the
        # Q7 stream, so it cannot deadlock and delays nothing urgent.
        if ch + 2 < nchunk:
            emit_x1_loads(ch + 2)
        for item in phases(ch):
            if item[0] == "dve":
                _, k, row0, nrows = item
                prod = prpool.tile([128, nrows, W], BF16, name="prod")
                nc.vector.tensor_mul(
                    prod[:, :, :],
                    x1bf[ch][:, row0 : row0 + nrows, :],
                    view_for(ch, k, row0, nrows),
                )
                emit_mms(ch, k, row0, nrows, prod)
            else:
                _, k, h = item
                emit_mms(ch, k, 48 * h, 48, prods[(k, h)])

    outT = [outpool.tile([32, 512], F32, name=f"outT{t}") for t in range(6)]
    for q in bank_completion:
        nc.scalar.mul(outT[q][:, :], ps[q][:, :], 1.0 / (128 * nchunk))

    outf = out_t.ap()  # [9, 96, 96] f32 DRAM
    out_flat = outf.rearrange("k a b -> k (a b)")
    for k in range(9):
        # rows 18k..18k+17 may span two banks; DMA each segment.
        m0 = 18 * k
        seg_start = 0
        while seg_start < 18:
            m = m0 + seg_start
            q, r = m // 32, m % 32
            cnt = min(18 - seg_start, 32 - r)
            nc.sync.dma_start(
                out=out_flat[k, seg_start * 512 : (seg_start + cnt) * 512],
                in_=outT[q][r : r + cnt, :],
            )
            seg_start += cnt


_CACHE = {}


def _build(c=C, debug=False):
    key = ("nc", c, os.environ.get("CORR_POOL_MULS", "7"))
    if key in _CACHE:
        return _CACHE[key]
    nchunk = c // 128
    nc = bacc.Bacc("TRN2", target_bir_lowering=False, debug=debug)
    x1_t = nc.dram_tensor("x_1", [c, H, W], F32, kind="ExternalInput")
    x2_t = nc.dram_tensor("x_2", [c, H, W], F32, kind="ExternalInput")
    out_t = nc.dram_tensor("out", [9, H, W], F32, kind="ExternalOutput")
    with tile.TileContext(nc) as tc, ExitStack() as ctx:
        _corr_body(ctx, tc, out_t, x1_t, x2_t, nchunk=nchunk)
    nc.compile()
    _CACHE[key] = nc
    return nc


def kernel(x_1: np.ndarray, x_2: np.ndarray) -> np.ndarray:
    x_1 = np.ascontiguousarray(np.asarray(x_1), dtype=np.float32)
    x_2 = np.ascontiguousarray(np.asarray(x_2), dtype=np.float32)
    assert x_1.shape == (B, C, H, W) and x_2.shape == (B, C, H, W)
    nc = _build()
    in_maps = [
        {"x_1": x_1[i].copy(), "x_2": x_2[i].copy()} for i in range(NCORES)
    ]
    last_err = None
    for attempt in range(3):
        try:
            res = run_bass_kernel_spmd(nc, in_maps, list(range(NCORES)))
            out = np.stack([res.results[i]["out"] for i in range(NCORES)], axis=0)
            return out.astype(np.float32)
        except Exception as e:  # rare transient device faults — retry
            last_err = e
            import time as _time

            _time.sleep(5.0 * (attempt + 1))
    raise last_err


if __name__ == "__main__":
    rng = np.random.default_rng(0)
    a = rng.standard_normal((B, C, H, W), dtype=np.float32)
    b = rng.standard_normal((B, C, H, W), dtype=np.float32)
    o = kernel(a, b)
    print("out", o.shape, o.dtype, float(np.abs(o).max()))
